# revision 20
# baseline (speedup 1.0000x reference)
"""Trainium2 Bass kernel for nn_Block_1563368095940 (sparse path attention block).

Strategy (8 NeuronCores, SPMD):
  feat is sharded across cores and AllGathered on device (cuts host->device
  transfer 8x vs replication).  cos/sin rows are pre-gathered on host into the
  per-core padded-window layout, so phase 1 only needs one contiguous DMA per
  group plus the feat-row indirect gather.  LN affine params and the 1/sqrt(D)
  q-scale are folded into the QKV / FC1 weights on host.

  Phase 1 (window-parallel): each core owns wpc=ceil(Wn/8) windows, processed
  in groups of 16 windows (8 blocks x 2 windows x 48 rows).  LN1 + QKV + rope
  + windowed attention in the S^T formulation (lhsT=expS_T, rhs=[V|1] yields
  outputs and softmax denominators in one matmul).  Attention rows are written
  contiguously (uncompacted) to a per-core stripe; stripes are AllGathered
  into a full table.

  Phase 2 (point-parallel): each core owns N/8 points.  Per 128-point group it
  gathers source rows from the table (host-precomputed position lists),
  segment-sums them with a one-hot binning matmul in PSUM, applies 1/cnt, the
  output projection, residual, LN2 and the MLP.  The core outputs
  delta = y - feat; the full-precision feat residual is added back on host.

  Both phase loops run as For_i hardware loops (program ~400 instructions
  instead of ~22k), which cuts bass+walrus compile time from ~7s to <1s.

Host-side work is limited to int index manipulation and layout/weight prep;
all heavy float tensor math runs on device.
"""

import math
import os
import sys

import numpy as np

for _p in ("/opt/trn_rl_repo", "/root/.axon_site/_ro/trn_rl_repo"):
    if _p not in sys.path:
        sys.path.insert(0, _p)

NCORES = 8
FORI = os.environ.get("KERNEL_FORI", "1") == "1"


def _build_program(shapes):
    import concourse.bass as bass
    import concourse.bacc as bacc
    import concourse.tile as tile
    from concourse import mybir
    from concourse.bass import ds, ts
    from concourse.masks import make_identity

    N, C, H, D, K, G1, NG2, W2 = (
        shapes["N"], shapes["C"], shapes["H"], shapes["D"],
        shapes["K"], shapes["G1"], shapes["NG2"], shapes["W2"],
    )
    f32 = mybir.dt.float32
    bf = mybir.dt.bfloat16
    f8 = mybir.dt.float8e4
    i32 = mybir.dt.int32
    BPG = shapes["BPG"]  # blocks per group (phase 1)
    BW = 2            # windows per block
    RPB = BW * K      # 96 rows per block
    RPG = BPG * RPB   # 768 rows per group
    NSL = N // NCORES  # points per core
    SROWS = G1 * RPG   # stripe rows per core

    nc = bacc.Bacc("TRN2", target_bir_lowering=False, num_devices=NCORES)

    # ---------------- I/O ----------------
    featsh_d = nc.dram_tensor("featsh", [NSL, C], f8, kind="ExternalInput")
    cs_d = nc.dram_tensor("cs", [G1 * RPB, BPG * 16], f8, kind="ExternalInput")
    gofs_d = nc.dram_tensor("gofs", [G1 * RPB, BPG], i32, kind="ExternalInput")
    wqkvT_d = nc.dram_tensor("wqkvT", [C, 3 * C], f32, kind="ExternalInput")
    qkvb_rep_d = nc.dram_tensor("qkvb_rep", [128, 3 * C], f32, kind="ExternalInput")
    projwT_d = nc.dram_tensor("projwT", [C, C], f32, kind="ExternalInput")
    projb_rep_d = nc.dram_tensor("projb_rep", [128, C], f32, kind="ExternalInput")
    fc1wT_d = nc.dram_tensor("fc1wT", [C, 4 * C], f32, kind="ExternalInput")
    fc1b_cols_d = nc.dram_tensor("fc1b_cols", [128, 4], f32, kind="ExternalInput")
    fc2wTT_d = nc.dram_tensor("fc2wTT", [4, 128, C], f32, kind="ExternalInput")
    fc2b_rep_d = nc.dram_tensor("fc2b_rep", [128, C], f32, kind="ExternalInput")
    bdmask_d = nc.dram_tensor("bdmask", [RPB, RPB], f32, kind="ExternalInput")
    iota_d = nc.dram_tensor("iota", [128, 128], bf, kind="ExternalInput")
    mh_d = nc.dram_tensor("mh", [128, H], f32, kind="ExternalInput")
    gofs2_d = nc.dram_tensor("gofs2", [NG2 * 128, W2], i32, kind="ExternalInput")
    tids_d = nc.dram_tensor("tids", [NG2 * 128, W2 + 1], bf, kind="ExternalInput")
    delta_d = nc.dram_tensor("delta", [NSL, C], f8, kind="ExternalOutput")

    featsh_i = nc.dram_tensor("featsh_i", [NSL, C], f8, kind="Internal")
    feat_full = nc.dram_tensor("feat_full", [N, C], f8, kind="Internal",
                               addr_space="Shared")
    stripe = nc.dram_tensor("stripe", [SROWS, C], bf, kind="Internal")
    table = nc.dram_tensor("table", [NCORES * SROWS, C], bf, kind="Internal",
                           addr_space="Shared")

    AX = mybir.AxisListType.X
    AF = mybir.ActivationFunctionType
    AL = mybir.AluOpType

    with tile.TileContext(nc) as tc:
        with (
            tc.tile_pool(name="singles", bufs=1) as singles,
            tc.tile_pool(name="sb", bufs=2) as sb,
        ):
            # AllGather the feat shards first so it overlaps weight loads.
            # (Collectives can't read IO tensors; stage through an Internal.)
            nc.sync.dma_start(out=featsh_i[:, :], in_=featsh_d[:, :])
            nc.gpsimd.collective_compute(
                "AllGather", AL.bypass,
                replica_groups=[list(range(NCORES))],
                ins=[featsh_i[:, :]], outs=[feat_full[:, :]])

            # ------------- constants / weights (once) -------------
            ident = singles.tile([128, 128], f32)
            make_identity(nc, ident[:])
            iota_sb = singles.tile([128, 128], bf)
            nc.sync.dma_start(out=iota_sb[:], in_=iota_d[:, :])
            bdm = singles.tile([RPB, RPB], f32)
            nc.sync.dma_start(out=bdm[:], in_=bdmask_d[:, :])
            mh = singles.tile([128, H], f32)
            nc.sync.dma_start(out=mh[:], in_=mh_d[:, :])
            wqkvT = singles.tile([C, 3 * C], f32)
            nc.sync.dma_start(out=wqkvT[:], in_=wqkvT_d[:, :])
            qkvb_rep = singles.tile([128, 3 * C], f32)
            nc.sync.dma_start(out=qkvb_rep[:], in_=qkvb_rep_d[:, :])
            projwT = singles.tile([C, C], f32)
            nc.sync.dma_start(out=projwT[:], in_=projwT_d[:, :])
            projb_rep = singles.tile([128, C], f32)
            nc.sync.dma_start(out=projb_rep[:], in_=projb_rep_d[:, :])
            fc1wT = singles.tile([C, 4 * C], f32)
            nc.sync.dma_start(out=fc1wT[:], in_=fc1wT_d[:, :])
            fc1bias = singles.tile([128, 4], f32)
            nc.sync.dma_start(out=fc1bias[:], in_=fc1b_cols_d[:, :])
            fc2wTT = singles.tile([128, 4, C], f32)
            nc.sync.dma_start(out=fc2wTT[:], in_=fc2wTT_d.rearrange("j f o -> f j o"))
            fc2b_rep = singles.tile([128, C], f32)
            nc.sync.dma_start(out=fc2b_rep[:], in_=fc2b_rep_d[:, :])
            eps96 = singles.tile([RPB, 1], f32)
            nc.vector.memset(eps96[:], 1e-5)
            eps128 = singles.tile([128, 1], f32)
            nc.vector.memset(eps128[:], 1e-5)

            # ---------------- Phase 1 ----------------
            pp1_cm = tc.tile_pool(name="pp1", bufs=1, space="PSUM")
            pp = pp1_cm.__enter__()

            def p1_body(g):
                gofs_t = sb.tile([RPB, BPG], i32)
                nc.sync.dma_start(out=gofs_t[:], in_=gofs_d[ts(g, RPB)])
                cstb = sb.tile([RPB, BPG * 16], f8)
                nc.sync.dma_start(out=cstb[:], in_=cs_d[ts(g, RPB)])
                cst = sb.tile([RPB, BPG * 16], f32)
                nc.vector.tensor_copy(out=cst[:], in_=cstb[:])

                fg8 = sb.tile([RPB, BPG, C], f8)
                for j in range(BPG):
                    nc.gpsimd.indirect_dma_start(
                        out=fg8[:, j, :], out_offset=None, in_=feat_full[:, :],
                        in_offset=bass.IndirectOffsetOnAxis(ap=gofs_t[:, j:j + 1], axis=0))
                fg = sb.tile([RPB, BPG, C], f32)
                nc.vector.tensor_copy(out=fg[:, :, :], in_=fg8[:, :, :])

                # LN1 stats per row (over C), batched across the 8 blocks
                ssum = sb.tile([RPB, BPG], f32)
                nc.vector.reduce_sum(out=ssum[:], in_=fg[:, :, :], axis=AX)
                mean = sb.tile([RPB, BPG], f32)
                nc.scalar.activation(out=mean[:], in_=ssum[:], func=AF.Copy,
                                     bias=0.0, scale=1.0 / C)
                hb = sb.tile([RPB, BPG, C], f32)
                nc.vector.tensor_tensor(out=hb[:, :, :], in0=fg[:, :, :],
                                        in1=fg[:, :, :], op=AL.mult)
                ssq = sb.tile([RPB, BPG], f32)
                nc.vector.reduce_sum(out=ssq[:], in_=hb[:, :, :], axis=AX)
                ex2 = sb.tile([RPB, BPG], f32)
                nc.scalar.activation(out=ex2[:], in_=ssq[:], func=AF.Copy,
                                     bias=0.0, scale=1.0 / C)
                msq = sb.tile([RPB, BPG], f32)
                nc.vector.tensor_tensor(out=msq[:], in0=mean[:], in1=mean[:], op=AL.mult)
                var = sb.tile([RPB, BPG], f32)
                nc.vector.tensor_tensor(out=var[:], in0=ex2[:], in1=msq[:], op=AL.subtract)
                rstd = sb.tile([RPB, BPG], f32)
                nc.scalar.activation(out=rstd[:], in_=var[:], func=AF.Sqrt,
                                     bias=eps96[:], scale=1.0)
                nc.vector.reciprocal(out=rstd[:], in_=rstd[:])

                # normalize per block, transpose to hT (hb reused as scratch)
                hT = sb.tile([128, RPG], f32)
                for j in range(BPG):
                    nc.vector.tensor_scalar(
                        out=hb[:, j, :], in0=fg[:, j, :],
                        scalar1=mean[:, j:j + 1], scalar2=rstd[:, j:j + 1],
                        op0=AL.subtract, op1=AL.mult)
                    tp = pp.tile([128, RPB], f32, tag="tr")
                    nc.tensor.transpose(tp[:], hb[:, j, :], ident[:RPB, :RPB])
                    nc.vector.tensor_copy(out=hT[:, j * RPB:(j + 1) * RPB], in_=tp[:])

                # QKV (rows layout) + bias; q-scale folded into weights
                qkv = sb.tile([RPB, BPG, 3 * C], f32)
                for j in range(BPG):
                    qkvp = pp.tile([RPB, 3 * C], f32, tag="qkvp")
                    nc.tensor.matmul(qkvp[:], lhsT=hT[:, j * RPB:(j + 1) * RPB],
                                     rhs=wqkvT[:], start=True, stop=True)
                    nc.vector.tensor_tensor(out=qkv[:, j, :], in0=qkvp[:],
                                            in1=qkvb_rep[:RPB, :], op=AL.add)

                # rope, batched over blocks via strided views
                ta = sb.tile([RPB, BPG * 64], f32)
                tb = sb.tile([RPB, BPG * 64], f32)
                tcs = sb.tile([RPB, BPG * 64], f32)

                def halves(base):
                    x1 = bass.AP(tensor=qkv.tensor, offset=qkv[:, 0, base].offset,
                                 ap=[qkv[:, 0, 0].ap[0],
                                     [3 * C, BPG], [D, H], [1, D // 2]])
                    x2 = bass.AP(tensor=qkv.tensor,
                                 offset=qkv[:, 0, base + D // 2].offset,
                                 ap=[qkv[:, 0, 0].ap[0],
                                     [3 * C, BPG], [D, H], [1, D // 2]])
                    return x1, x2

                def cs_view(col0):
                    return bass.AP(tensor=cst.tensor,
                                   offset=cst[:, col0].offset,
                                   ap=[cst[:, 0].ap[0], [16, BPG], [0, H],
                                       [1, D // 2]])

                def t3(t):
                    return bass.AP(tensor=t.tensor, offset=t[:, 0].offset,
                                   ap=[t[:, 0].ap[0], [64, BPG], [8, H], [1, 8]])

                cb, sbv = cs_view(0), cs_view(8)
                for base in (0, C):
                    x1, x2 = halves(base)
                    nc.vector.tensor_tensor(out=t3(ta), in0=x1, in1=cb, op=AL.mult)
                    nc.vector.tensor_tensor(out=t3(tb), in0=x1, in1=sbv, op=AL.mult)
                    nc.vector.tensor_tensor(out=t3(tcs), in0=x2, in1=sbv, op=AL.mult)
                    nc.vector.tensor_tensor(out=x1, in0=t3(ta), in1=t3(tcs), op=AL.subtract)
                    nc.vector.tensor_tensor(out=t3(ta), in0=x2, in1=cb, op=AL.mult)
                    nc.vector.tensor_tensor(out=x2, in0=t3(ta), in1=t3(tb), op=AL.add)

                # transpose q,k to T layout; build v1 = [V|1] per head
                qT = sb.tile([128, RPG], f32)
                kT = sb.tile([128, RPG], f32)
                v1 = sb.tile([RPB, BPG, H * (D + 1)], f32)
                for j in range(BPG):
                    tq = pp.tile([128, RPB], f32, tag="tr")
                    nc.tensor.transpose(tq[:], qkv[:, j, 0:C], ident[:RPB, :RPB])
                    nc.vector.tensor_copy(out=qT[:, j * RPB:(j + 1) * RPB], in_=tq[:])
                    tk = pp.tile([128, RPB], f32, tag="tr")
                    nc.tensor.transpose(tk[:], qkv[:, j, C:2 * C], ident[:RPB, :RPB])
                    nc.vector.tensor_copy(out=kT[:, j * RPB:(j + 1) * RPB], in_=tk[:])
                    vdst = bass.AP(tensor=v1.tensor, offset=v1[:, j, 0].offset,
                                   ap=[v1[:, 0, 0].ap[0], [D + 1, H], [1, D]])
                    vsrc = bass.AP(tensor=qkv.tensor, offset=qkv[:, j, 2 * C].offset,
                                   ap=[qkv[:, 0, 0].ap[0], [D, H], [1, D]])
                    nc.vector.tensor_copy(out=vdst, in_=vsrc)
                    onesd = bass.AP(tensor=v1.tensor, offset=v1[:, j, D].offset,
                                    ap=[v1[:, 0, 0].ap[0], [D + 1, H], [1, 1]])
                    nc.gpsimd.memset(onesd, 1.0)

                qTh = sb.tile([128, H, RPG], f32)
                for h in range(H):
                    nc.vector.tensor_scalar_mul(out=qTh[:, h, :], in0=qT[:],
                                                scalar1=mh[:, h:h + 1])

                # attention per block
                osb = sb.tile([RPB, BPG, C], bf)
                for j in range(BPG):
                    expS = sb.tile([RPB, H, RPB], f32)
                    for m in range(2):
                        sB = pp.tile([RPB, 4 * RPB], f32, tag="sB%d" % m)
                        for hh in range(4):
                            h = 4 * m + hh
                            nc.tensor.matmul(
                                sB[:, hh * RPB:(hh + 1) * RPB],
                                lhsT=kT[:, j * RPB:(j + 1) * RPB],
                                rhs=qTh[:, h, j * RPB:(j + 1) * RPB],
                                start=True, stop=True)
                        etmp = sb.tile([RPB, 4 * RPB], f32, tag="etmp")
                        nc.scalar.activation(out=etmp[:], in_=sB[:], func=AF.Exp)
                        mview = bass.AP(tensor=bdm.tensor, offset=bdm[:, 0].offset,
                                        ap=[bdm[:, 0].ap[0], [0, 4], [1, RPB]])
                        nc.vector.tensor_tensor(
                            out=expS[:, 4 * m:4 * (m + 1), :],
                            in0=etmp[:].rearrange("p (h r) -> p h r", h=4),
                            in1=mview, op=AL.mult)
                    oP = pp.tile([RPB, H * (D + 1)], f32, tag="oP")
                    for h in range(H):
                        nc.tensor.matmul(
                            oP[:, h * (D + 1):(h + 1) * (D + 1)],
                            lhsT=expS[:, h, :],
                            rhs=v1[:, j, h * (D + 1):(h + 1) * (D + 1)],
                            start=True, stop=True)
                    rec = sb.tile([RPB, H], f32)
                    dview = bass.AP(tensor=oP.tensor, offset=oP[:, D].offset,
                                    ap=[oP[:, 0].ap[0], [D + 1, H], [1, 1]])
                    nc.vector.reciprocal(out=rec[:].rearrange("p (h o) -> p h o", h=H),
                                         in_=dview)
                    oview = bass.AP(tensor=oP.tensor, offset=oP[:, 0].offset,
                                    ap=[oP[:, 0].ap[0], [D + 1, H], [1, D]])
                    rview = bass.AP(tensor=rec.tensor, offset=rec[:, 0].offset,
                                    ap=[rec[:, 0].ap[0], [1, H], [0, D]])
                    nc.vector.tensor_tensor(
                        out=osb[:, j, :].rearrange("p (h d) -> p h d", h=H),
                        in0=oview, in1=rview, op=AL.mult)

                # contiguous stripe write: stripe row = g*RPG + j*RPB + p
                nc.sync.dma_start(
                    out=stripe[ts(g, RPG)].rearrange("(j p) c -> p j c", j=BPG),
                    in_=osb[:, :, :])

            if not os.environ.get("SKIP_P1"):
                if FORI:
                    with tc.For_i(0, G1, 1) as gv:
                        p1_body(gv)
                else:
                    for g in range(G1):
                        p1_body(g)
            pp1_cm.__exit__(None, None, None)

            # ---------------- AllGather ----------------
            nc.gpsimd.collective_compute(
                "AllGather", AL.bypass,
                replica_groups=[list(range(NCORES))],
                ins=[stripe[:, :]], outs=[table[:, :]])

            # ---------------- Phase 2 ----------------
            pp2_cm = tc.tile_pool(name="pp2", bufs=2, space="PSUM")
            pp2 = pp2_cm.__enter__()

            def p2_body(g):
                g2o = sb.tile([128, W2], i32)
                nc.sync.dma_start(out=g2o[:], in_=gofs2_d[ts(g, 128)])
                tid = sb.tile([128, W2 + 1], bf)
                nc.sync.dma_start(out=tid[:], in_=tids_d[ts(g, 128)])
                gath = sb.tile([128, W2, C], bf)
                for j in range(W2):
                    nc.gpsimd.indirect_dma_start(
                        out=gath[:, j, :], out_offset=None, in_=table[:, :],
                        in_offset=bass.IndirectOffsetOnAxis(ap=g2o[:, j:j + 1], axis=0))

                binp = pp2.tile([128, C], f32, tag="mm2")
                for j in range(W2):
                    oh = sb.tile([128, 128], bf, tag="oh")
                    nc.vector.tensor_tensor(out=oh[:], in0=tid[:, j:j + 1].to_broadcast([128, 128]),
                                            in1=iota_sb[:], op=AL.is_equal)
                    nc.tensor.matmul(binp[:], lhsT=oh[:], rhs=gath[:, j, :],
                                     start=(j == 0), stop=(j == W2 - 1))
                invf = sb.tile([128, 1], f32, tag="invf")
                nc.vector.tensor_copy(out=invf[:], in_=tid[:, W2:W2 + 1])
                a_sb = sb.tile([128, C], f32)
                nc.vector.tensor_scalar_mul(out=a_sb[:], in0=binp[:],
                                            scalar1=invf[:])
                tpa = pp2.tile([128, 128], f32, tag="tr2")
                nc.tensor.transpose(tpa[:], a_sb[:], ident[:])
                aT = sb.tile([128, 128], f32)
                nc.vector.tensor_copy(out=aT[:], in_=tpa[:])
                prp = pp2.tile([128, C], f32, tag="mm2")
                nc.tensor.matmul(prp[:], lhsT=aT[:], rhs=projwT[:], start=True, stop=True)

                asum = sb.tile([128, C], f32)
                nc.vector.tensor_tensor(out=asum[:], in0=prp[:], in1=projb_rep[:], op=AL.add)
                feattb = sb.tile([128, C], f8)
                nc.sync.dma_start(out=feattb[:], in_=featsh_d[ts(g, 128)])
                featt = sb.tile([128, C], f32)
                nc.vector.tensor_copy(out=featt[:], in_=feattb[:])
                x_sb = sb.tile([128, C], f32)
                nc.vector.tensor_tensor(out=x_sb[:], in0=asum[:], in1=featt[:], op=AL.add)

                # LN2
                s1 = sb.tile([128, 1], f32, tag="s1")
                nc.vector.reduce_sum(out=s1[:], in_=x_sb[:], axis=AX)
                mn = sb.tile([128, 1], f32, tag="mn")
                nc.scalar.activation(out=mn[:], in_=s1[:], func=AF.Copy, bias=0.0,
                                     scale=1.0 / C)
                sqx = sb.tile([128, C], f32, tag="sqx")
                nc.vector.tensor_tensor(out=sqx[:], in0=x_sb[:], in1=x_sb[:], op=AL.mult)
                s2 = sb.tile([128, 1], f32, tag="s2")
                nc.vector.reduce_sum(out=s2[:], in_=sqx[:], axis=AX)
                e2 = sb.tile([128, 1], f32, tag="e2")
                nc.scalar.activation(out=e2[:], in_=s2[:], func=AF.Copy, bias=0.0,
                                     scale=1.0 / C)
                m2t = sb.tile([128, 1], f32, tag="m2t")
                nc.vector.tensor_tensor(out=m2t[:], in0=mn[:], in1=mn[:], op=AL.mult)
                vr = sb.tile([128, 1], f32, tag="vr")
                nc.vector.tensor_tensor(out=vr[:], in0=e2[:], in1=m2t[:], op=AL.subtract)
                rs = sb.tile([128, 1], f32, tag="rs")
                nc.scalar.activation(out=rs[:], in_=vr[:], func=AF.Sqrt,
                                     bias=eps128[:], scale=1.0)
                nc.vector.reciprocal(out=rs[:], in_=rs[:])
                h2 = sb.tile([128, C], f32, tag="h2")
                nc.vector.tensor_scalar(out=h2[:], in0=x_sb[:], scalar1=mn[:],
                                        scalar2=rs[:], op0=AL.subtract, op1=AL.mult)
                tph = pp2.tile([128, 128], f32, tag="tr2")
                nc.tensor.transpose(tph[:], h2[:], ident[:])
                h2T = sb.tile([128, 128], f32, tag="h2T")
                nc.vector.tensor_copy(out=h2T[:], in_=tph[:])

                gT = sb.tile([128, 4, 128], f32, tag="gT")
                for j in range(4):
                    f1p = pp2.tile([128, 128], f32, tag="f1p")
                    nc.tensor.matmul(f1p[:], lhsT=fc1wT[:, j * 128:(j + 1) * 128],
                                     rhs=h2T[:], start=True, stop=True)
                    nc.scalar.activation(out=gT[:, j, :], in_=f1p[:], func=AF.Gelu,
                                         bias=fc1bias[:, j:j + 1], scale=1.0)
                f2p = pp2.tile([128, C], f32, tag="mm2")
                for j in range(4):
                    nc.tensor.matmul(f2p[:], lhsT=gT[:, j, :], rhs=fc2wTT[:, j, :],
                                     start=(j == 0), stop=(j == 3))
                dsb = sb.tile([128, C], f8, tag="dsb")
                nc.vector.tensor_tensor(out=dsb[:], in0=f2p[:], in1=asum[:], op=AL.add)
                nc.vector.tensor_tensor(out=dsb[:], in0=dsb[:], in1=fc2b_rep[:], op=AL.add)
                nc.sync.dma_start(out=delta_d[ts(g, 128)], in_=dsb[:])

            if not os.environ.get("SKIP_P2"):
                if FORI:
                    with tc.For_i(0, NG2, 1) as gv2:
                        p2_body(gv2)
                else:
                    for g in range(NG2):
                        p2_body(g)
            else:
                zt = sb.tile([128, C], f8, tag="dsb")
                nc.vector.memset(zt[:], 0.0)
                for g in range(NG2):
                    nc.sync.dma_start(out=delta_d[ts(g, 128)], in_=zt[:])
            pp2_cm.__exit__(None, None, None)

    nc.compile()
    return nc


def _import_jax():
    import jax
    from concourse import bass2jax  # noqa: F401
    try:
        devs = jax.devices()
        # Warm backend init + the axon transfer path (no compilation).
        jax.device_put(np.ones((8, 8), np.float32), devs[0]).block_until_ready()
        import libneuronxla  # noqa: F401
    except Exception:
        pass


def _import_worker():
    # Warm the heavy imports while the main thread does numpy index prep,
    # then trace+schedule a tiny throwaway program so the first real build
    # hits warm code paths (bass_rust engines, tile scheduler, masks, ...).
    import concourse.bass_utils  # noqa: F401
    import concourse.bacc as bacc
    import concourse.tile as tile
    from concourse import mybir  # noqa: F401
    from concourse.masks import make_identity
    nc = bacc.Bacc("TRN2", target_bir_lowering=False, num_devices=NCORES)
    x_d = nc.dram_tensor("x", [128, 128], mybir.dt.float32, kind="ExternalInput")
    y_d = nc.dram_tensor("y", [128, 128], mybir.dt.float32, kind="ExternalOutput")
    with tile.TileContext(nc) as tc:
        with tc.tile_pool(name="sb", bufs=1) as sb:
            t = sb.tile([128, 128], mybir.dt.float32)
            make_identity(nc, t[:])
            nc.sync.dma_start(out=t[:], in_=x_d[:, :])
            with tc.For_i(0, 2, 1) as i:
                from concourse.bass import ts
                t2 = sb.tile([64, 128], mybir.dt.float32)
                nc.vector.tensor_scalar_mul(out=t2[:], in0=t[0:64, :], scalar1=t[0:64, 0:1])
                nc.sync.dma_start(out=y_d[ts(i, 64), :], in_=t2[:])
            nc.sync.dma_start(out=y_d[:, :], in_=t[:])
    nc.compile()
    nc.to_json_bytes()


def kernel(**inputs):
    import threading
    import time as _time
    _t0 = _time.time()
    _tlog = (lambda msg: print(f"[ktime] {msg}: {_time.time()-_t0:.2f}s", flush=True)) \
        if os.environ.get("KTIME") else (lambda msg: None)
    _imp = threading.Thread(target=_import_worker)
    _imp.start()
    _impj = threading.Thread(target=_import_jax)
    _impj.start()
    _tlog("import threads started")

    feat = np.ascontiguousarray(np.asarray(inputs["feat"], dtype=np.float32))
    cos = np.asarray(inputs["cos"], dtype=np.float32)
    sin = np.asarray(inputs["sin"], dtype=np.float32)
    pad = np.asarray(inputs["pad"]).astype(np.int64)
    unpad = np.asarray(inputs["unpad"]).astype(np.int64)
    pinv = np.asarray(inputs["path_inverse"]).astype(np.int64)
    H = int(inputs["num_heads"])
    K = int(inputs["patch_size"])
    N, C = feat.shape
    D = C // H
    M = pinv.shape[0]
    Mpad = pad.shape[0]
    Wn = Mpad // K
    assert Wn * K == Mpad and N % (128 * NCORES) == 0

    BW, BPG = 2, 2
    RPB = BW * K
    RPG = BPG * RPB
    WPG = BW * BPG                       # windows per phase-1 group
    wpc = (Wn + NCORES - 1) // NCORES    # windows per core
    G1 = (wpc + WPG - 1) // WPG
    SROWS = G1 * RPG
    NSL = N // NCORES
    NG2 = NSL // 128
    NGRP = N // 128

    gidx_all = pinv[pad]                           # [Mpad] -> feat row
    real_all = unpad[pad] == np.arange(Mpad)       # row is a real (unpadded) row

    # ---- phase-1 grids (vectorized) ----
    c_ = np.arange(NCORES)[:, None, None, None]
    g_ = np.arange(G1)[None, :, None, None]
    p_ = np.arange(RPB)[None, None, :, None]
    j_ = np.arange(BPG)[None, None, None, :]
    s_ = g_ * WPG + j_ * BW + p_ // K              # slot in core
    w_ = c_ * wpc + s_                             # global window
    mask = (s_ < wpc) & (w_ < Wn)
    mp_ = np.where(mask, w_ * K + p_ % K, 0)
    gofs = np.where(mask, gidx_all[mp_], 0).astype(np.int32)
    pid_ = pad[mp_]
    cos8 = np.ascontiguousarray(cos[:, :D // 2])
    sin8 = np.ascontiguousarray(sin[:, :D // 2])
    csq = np.empty((NCORES, G1, RPB, BPG, 16), np.float32)
    csq[..., 0:8] = cos8[pid_]
    csq[..., 8:16] = sin8[pid_]
    csq *= mask[..., None]
    _tlog("host p1 prep")

    # ---- phase-2: per 128-point group source lists ----
    mp_real = np.nonzero(real_all)[0]
    wr = mp_real // K
    rr = mp_real - wr * K
    cr = wr // wpc
    sr = wr - cr * wpc
    jjr = sr % WPG
    srow = (sr // WPG) * RPG + (jjr // BW) * RPB + (jjr % BW) * K + rr
    tblpos = cr * SROWS + srow
    tgt = gidx_all[mp_real]
    order = np.argsort(tgt, kind="stable")
    tgt_s, tbl_s = tgt[order], tblpos[order]
    starts = np.searchsorted(tgt_s, np.arange(0, N + 1, 128))
    cnt_grp = np.diff(starts)
    W2 = int(math.ceil(cnt_grp.max() / 128))
    grp = tgt_s // 128
    off = np.arange(tgt_s.shape[0]) - starts[grp]
    gof_flat = np.zeros((NGRP, 128 * W2), np.int32)
    tid_flat = np.full((NGRP, 128 * W2), -1.0, np.float32)
    gof_flat[grp, off] = tbl_s
    tid_flat[grp, off] = (tgt_s - grp * 128).astype(np.float32)
    gofs2 = gof_flat.reshape(NCORES, NG2 * 128, W2)
    cnts = np.bincount(pinv, minlength=N)
    invc = (1.0 / np.maximum(cnts, 1)).astype(np.float32)
    tids2 = np.concatenate(
        [tid_flat.reshape(NGRP, 128, W2), invc.reshape(NGRP, 128, 1)],
        axis=2).reshape(NCORES, NG2 * 128, W2 + 1)
    import ml_dtypes
    tids2 = tids2.astype(ml_dtypes.bfloat16)
    _tlog("host p2 prep")

    # ---- weights (LN affines + q-scale folded on host) ----
    g1 = np.asarray(inputs["g1"], np.float32)
    b1 = np.asarray(inputs["b1"], np.float32)
    g2 = np.asarray(inputs["g2"], np.float32)
    b2 = np.asarray(inputs["b2"], np.float32)
    qkv_w = np.asarray(inputs["qkv_w"], np.float32)
    qkv_b = np.asarray(inputs["qkv_b"], np.float32)
    proj_w = np.asarray(inputs["proj_w"], np.float32)
    fc1_w = np.asarray(inputs["fc1_w"], np.float32)
    fc1_b = np.asarray(inputs["fc1_b"], np.float32)
    fc2_w = np.asarray(inputs["fc2_w"], np.float32)
    qs = float(D) ** -0.5
    wqkvT = qkv_w.T * g1[:, None]
    wqkvT[:, 0:C] *= qs
    qkvb = b1 @ qkv_w.T + qkv_b
    qkvb[0:C] *= qs
    fc1wT = fc1_w.T * g2[:, None]
    fc1b = b2 @ fc1_w.T + fc1_b
    fc2wTT = np.stack([np.ascontiguousarray(fc2_w[:, j * 128:(j + 1) * 128].T)
                       for j in range(4)])
    common = {
        "wqkvT": np.ascontiguousarray(wqkvT),
        "qkvb_rep": np.tile(qkvb, (128, 1)),
        "projwT": np.ascontiguousarray(proj_w.T),
        "projb_rep": np.tile(np.asarray(inputs["proj_b"], np.float32), (128, 1)),
        "fc1wT": np.ascontiguousarray(fc1wT),
        "fc1b_cols": np.ascontiguousarray(fc1b.reshape(4, 128).T),
        "fc2wTT": fc2wTT,
        "fc2b_rep": np.tile(np.asarray(inputs["fc2_b"], np.float32), (128, 1)),
        "bdmask": np.kron(np.eye(BW, dtype=np.float32),
                          np.ones((K, K), np.float32)),
        "iota": np.tile(np.arange(128, dtype=np.float32), (128, 1)).astype(ml_dtypes.bfloat16),
        "mh": (np.arange(128)[:, None] // D == np.arange(H)[None, :]
               ).astype(np.float32),
    }
    shapes = dict(N=N, C=C, H=H, D=D, K=K, G1=G1, NG2=NG2, W2=W2, BPG=BPG)
    import ml_dtypes
    bf16 = ml_dtypes.bfloat16
    f8np = ml_dtypes.float8_e4m3
    featbf = feat.astype(f8np)
    csqbf = csq.astype(f8np)
    _tlog("input prep")
    _imp.join()
    _impj.join()
    _tlog("import join")
    from concourse.bass_utils import run_bass_kernel_spmd
    nc = _build_program(shapes)
    _tlog("build+compile bass")
    in_maps = []
    for c in range(NCORES):
        m = dict(common)
        m.update({
            "featsh": featbf[c * NSL:(c + 1) * NSL],
            "cs": csqbf[c].reshape(G1 * RPB, BPG * 16),
            "gofs": gofs[c].reshape(G1 * RPB, BPG),
            "gofs2": gofs2[c], "tids": tids2[c],
        })
        in_maps.append(m)
    _tlog("in_maps")
    res = run_bass_kernel_spmd(nc, in_maps, core_ids=list(range(NCORES)))
    _tlog("run_bass_kernel_spmd")
    outs = res.results
    y = np.empty_like(feat)
    for c in range(NCORES):
        sl = slice(c * NSL, (c + 1) * NSL)
        np.add(feat[sl], np.asarray(outs[c]["delta"], dtype=np.float32), out=y[sl])
    _tlog("gather output")
    return y


if __name__ == "__main__":
    sys.path.insert(0, "/root/problem")
    import reference
    inp = reference.setup_inputs()
    inp = {k: np.asarray(v) if hasattr(v, "shape") else v for k, v in inp.items()}
    out = kernel(**inp)
    print("kernel out", out.shape, out.dtype)


# revision 22
# speedup vs baseline: 1.1883x; 1.1883x over previous
"""Trainium2 Bass kernel for nn_Block_1563368095940 (sparse path attention block).

Strategy (8 NeuronCores, SPMD):
  feat is sharded across cores and AllGathered on device (cuts host->device
  transfer 8x vs replication).  cos/sin rows are pre-gathered on host into the
  per-core padded-window layout, so phase 1 only needs one contiguous DMA per
  group plus the feat-row indirect gather.  LN affine params and the 1/sqrt(D)
  q-scale are folded into the QKV / FC1 weights on host.

  Phase 1 (window-parallel): each core owns wpc=ceil(Wn/8) windows, processed
  in groups of 16 windows (8 blocks x 2 windows x 48 rows).  LN1 + QKV + rope
  + windowed attention in the S^T formulation (lhsT=expS_T, rhs=[V|1] yields
  outputs and softmax denominators in one matmul).  Attention rows are written
  contiguously (uncompacted) to a per-core stripe; stripes are AllGathered
  into a full table.

  Phase 2 (point-parallel): each core owns N/8 points.  Per 128-point group it
  gathers source rows from the table (host-precomputed position lists),
  segment-sums them with a one-hot binning matmul in PSUM, applies 1/cnt, the
  output projection, residual, LN2 and the MLP.  The core outputs
  delta = y - feat; the full-precision feat residual is added back on host.

  Both phase loops run as For_i hardware loops (program ~400 instructions
  instead of ~22k), which cuts bass+walrus compile time from ~7s to <1s.

Host-side work is limited to int index manipulation and layout/weight prep;
all heavy float tensor math runs on device.
"""

import math
import os
import sys

import numpy as np

for _p in ("/opt/trn_rl_repo", "/root/.axon_site/_ro/trn_rl_repo"):
    if _p not in sys.path:
        sys.path.insert(0, _p)

NCORES = 8
FORI = os.environ.get("KERNEL_FORI", "1") == "1"


def _build_program(shapes):
    import concourse.bass as bass
    import concourse.bacc as bacc
    import concourse.tile as tile
    from concourse import mybir
    from concourse.bass import ds, ts
    from concourse.masks import make_identity

    N, C, H, D, K, G1, NG2, W2 = (
        shapes["N"], shapes["C"], shapes["H"], shapes["D"],
        shapes["K"], shapes["G1"], shapes["NG2"], shapes["W2"],
    )
    f32 = mybir.dt.float32
    bf = mybir.dt.bfloat16
    f8 = mybir.dt.float8e4
    i32 = mybir.dt.int32
    BPG = shapes["BPG"]  # blocks per group (phase 1)
    BW = 2            # windows per block
    RPB = BW * K      # 96 rows per block
    RPG = BPG * RPB   # 768 rows per group
    NSL = N // NCORES  # points per core
    SROWS = G1 * RPG   # stripe rows per core

    nc = bacc.Bacc("TRN2", target_bir_lowering=False, num_devices=NCORES)

    # ---------------- I/O ----------------
    featsh_d = nc.dram_tensor("featsh", [NSL, C], f8, kind="ExternalInput")
    cs_d = nc.dram_tensor("cs", [G1 * RPB, BPG * 16], f8, kind="ExternalInput")
    gofs_d = nc.dram_tensor("gofs", [G1 * RPB, BPG], i32, kind="ExternalInput")
    TOTW = 128 * (3 * C + C + 4 * C + 4 * C) + (3 * C + C + C) + 128 * 4 \
        + 128 * H + RPB * RPB
    TOTW += (-TOTW) % NCORES
    wsh_d = nc.dram_tensor("wsh", [TOTW // NCORES], f32, kind="ExternalInput")
    iota_d = nc.dram_tensor("iota", [128, 128], bf, kind="ExternalInput")
    gofs2_d = nc.dram_tensor("gofs2", [NG2 * 128, W2], i32, kind="ExternalInput")
    tids_d = nc.dram_tensor("tids", [NG2 * 128, W2 + 1], bf, kind="ExternalInput")
    delta_d = nc.dram_tensor("delta", [NSL, C], f8, kind="ExternalOutput")

    featsh_i = nc.dram_tensor("featsh_i", [NSL, C], f8, kind="Internal")
    wsh_i = nc.dram_tensor("wsh_i", [TOTW // NCORES], f32, kind="Internal")
    wblob = nc.dram_tensor("wblob", [TOTW], f32, kind="Internal",
                           addr_space="Shared")
    feat_full = nc.dram_tensor("feat_full", [N, C], f8, kind="Internal",
                               addr_space="Shared")
    stripe = nc.dram_tensor("stripe", [SROWS, C], bf, kind="Internal")
    table = nc.dram_tensor("table", [NCORES * SROWS, C], bf, kind="Internal",
                           addr_space="Shared")

    AX = mybir.AxisListType.X
    AF = mybir.ActivationFunctionType
    AL = mybir.AluOpType

    with tile.TileContext(nc) as tc:
        with (
            tc.tile_pool(name="singles", bufs=1) as singles,
            tc.tile_pool(name="sb", bufs=2) as sb,
        ):
            # AllGather the feat + weight shards first so they overlap
            # the rest of the setup.  (Collectives can't read IO tensors;
            # stage through Internals.)
            nc.sync.dma_start(out=wsh_i[:], in_=wsh_d[:])
            nc.gpsimd.collective_compute(
                "AllGather", AL.bypass,
                replica_groups=[list(range(NCORES))],
                ins=[wsh_i[:]], outs=[wblob[:]])
            nc.sync.dma_start(out=featsh_i[:, :], in_=featsh_d[:, :])
            nc.gpsimd.collective_compute(
                "AllGather", AL.bypass,
                replica_groups=[list(range(NCORES))],
                ins=[featsh_i[:, :]], outs=[feat_full[:, :]])

            def wview(ofs, p, x):
                return wblob[ofs:ofs + p * x].rearrange("(p x) -> p x", p=p)

            # ------------- constants / weights (once) -------------
            ident = singles.tile([128, 128], f32)
            make_identity(nc, ident[:])
            iota_sb = singles.tile([128, 128], bf)
            nc.sync.dma_start(out=iota_sb[:], in_=iota_d[:, :])
            ofs = 0
            wqkvT = singles.tile([C, 3 * C], f32)
            nc.sync.dma_start(out=wqkvT[:], in_=wview(ofs, C, 3 * C)); ofs += C * 3 * C
            projwT = singles.tile([C, C], f32)
            nc.sync.dma_start(out=projwT[:], in_=wview(ofs, C, C)); ofs += C * C
            fc1wT = singles.tile([C, 4 * C], f32)
            nc.sync.dma_start(out=fc1wT[:], in_=wview(ofs, C, 4 * C)); ofs += C * 4 * C
            fc2wTT = singles.tile([128, 4, C], f32)
            nc.sync.dma_start(out=fc2wTT[:], in_=wview(ofs, 128, 4 * C)); ofs += 128 * 4 * C
            brows = singles.tile([1, 3 * C + 2 * C], f32)
            nc.sync.dma_start(out=brows[:], in_=wblob[ofs:ofs + 5 * C].rearrange("(p x) -> p x", p=1))
            ofs += 5 * C
            fc1bias = singles.tile([128, 4], f32)
            nc.sync.dma_start(out=fc1bias[:], in_=wview(ofs, 128, 4)); ofs += 128 * 4
            mh = singles.tile([128, H], f32)
            nc.sync.dma_start(out=mh[:], in_=wview(ofs, 128, H)); ofs += 128 * H
            bdm = singles.tile([RPB, RPB], f32)
            nc.sync.dma_start(out=bdm[:], in_=wview(ofs, RPB, RPB)); ofs += RPB * RPB
            # replicate bias rows across partitions via ones-column matmul
            ones1 = singles.tile([1, 128], f32)
            nc.vector.memset(ones1[:], 1.0)
            ppw_cm = tc.tile_pool(name="ppw", bufs=1, space="PSUM")
            ppw = ppw_cm.__enter__()
            qkvb_rep = singles.tile([128, 3 * C], f32)
            bp1 = ppw.tile([128, 3 * C], f32, tag="brep")
            nc.tensor.matmul(bp1[:], lhsT=ones1[:], rhs=brows[:, 0:3 * C],
                             start=True, stop=True)
            nc.vector.tensor_copy(out=qkvb_rep[:], in_=bp1[:])
            projb_rep = singles.tile([128, C], f32)
            bp2 = ppw.tile([128, 3 * C], f32, tag="brep")
            nc.tensor.matmul(bp2[:, 0:C], lhsT=ones1[:], rhs=brows[:, 3 * C:4 * C],
                             start=True, stop=True)
            nc.vector.tensor_copy(out=projb_rep[:], in_=bp2[:, 0:C])
            fc2b_rep = singles.tile([128, C], f32)
            bp3 = ppw.tile([128, 3 * C], f32, tag="brep")
            nc.tensor.matmul(bp3[:, 0:C], lhsT=ones1[:], rhs=brows[:, 4 * C:5 * C],
                             start=True, stop=True)
            nc.vector.tensor_copy(out=fc2b_rep[:], in_=bp3[:, 0:C])
            ppw_cm.__exit__(None, None, None)
            eps96 = singles.tile([RPB, 1], f32)
            nc.vector.memset(eps96[:], 1e-5)
            eps128 = singles.tile([128, 1], f32)
            nc.vector.memset(eps128[:], 1e-5)

            # ---------------- Phase 1 ----------------
            pp1_cm = tc.tile_pool(name="pp1", bufs=1, space="PSUM")
            pp = pp1_cm.__enter__()

            def p1_body(g):
                gofs_t = sb.tile([RPB, BPG], i32)
                nc.sync.dma_start(out=gofs_t[:], in_=gofs_d[ts(g, RPB)])
                cstb = sb.tile([RPB, BPG * 16], f8)
                nc.sync.dma_start(out=cstb[:], in_=cs_d[ts(g, RPB)])
                cst = sb.tile([RPB, BPG * 16], f32)
                nc.vector.tensor_copy(out=cst[:], in_=cstb[:])

                fg8 = sb.tile([RPB, BPG, C], f8)
                for j in range(BPG):
                    nc.gpsimd.indirect_dma_start(
                        out=fg8[:, j, :], out_offset=None, in_=feat_full[:, :],
                        in_offset=bass.IndirectOffsetOnAxis(ap=gofs_t[:, j:j + 1], axis=0))
                fg = sb.tile([RPB, BPG, C], f32)
                nc.vector.tensor_copy(out=fg[:, :, :], in_=fg8[:, :, :])

                # LN1 stats per row (over C), batched across the 8 blocks
                ssum = sb.tile([RPB, BPG], f32)
                nc.vector.reduce_sum(out=ssum[:], in_=fg[:, :, :], axis=AX)
                mean = sb.tile([RPB, BPG], f32)
                nc.scalar.activation(out=mean[:], in_=ssum[:], func=AF.Copy,
                                     bias=0.0, scale=1.0 / C)
                hb = sb.tile([RPB, BPG, C], f32)
                nc.vector.tensor_tensor(out=hb[:, :, :], in0=fg[:, :, :],
                                        in1=fg[:, :, :], op=AL.mult)
                ssq = sb.tile([RPB, BPG], f32)
                nc.vector.reduce_sum(out=ssq[:], in_=hb[:, :, :], axis=AX)
                ex2 = sb.tile([RPB, BPG], f32)
                nc.scalar.activation(out=ex2[:], in_=ssq[:], func=AF.Copy,
                                     bias=0.0, scale=1.0 / C)
                msq = sb.tile([RPB, BPG], f32)
                nc.vector.tensor_tensor(out=msq[:], in0=mean[:], in1=mean[:], op=AL.mult)
                var = sb.tile([RPB, BPG], f32)
                nc.vector.tensor_tensor(out=var[:], in0=ex2[:], in1=msq[:], op=AL.subtract)
                rstd = sb.tile([RPB, BPG], f32)
                nc.scalar.activation(out=rstd[:], in_=var[:], func=AF.Sqrt,
                                     bias=eps96[:], scale=1.0)
                nc.vector.reciprocal(out=rstd[:], in_=rstd[:])

                # normalize per block, transpose to hT (hb reused as scratch)
                hT = sb.tile([128, RPG], f32)
                for j in range(BPG):
                    nc.vector.tensor_scalar(
                        out=hb[:, j, :], in0=fg[:, j, :],
                        scalar1=mean[:, j:j + 1], scalar2=rstd[:, j:j + 1],
                        op0=AL.subtract, op1=AL.mult)
                    tp = pp.tile([128, RPB], f32, tag="tr")
                    nc.tensor.transpose(tp[:], hb[:, j, :], ident[:RPB, :RPB])
                    nc.vector.tensor_copy(out=hT[:, j * RPB:(j + 1) * RPB], in_=tp[:])

                # QKV (rows layout) + bias; q-scale folded into weights
                qkv = sb.tile([RPB, BPG, 3 * C], f32)
                for j in range(BPG):
                    qkvp = pp.tile([RPB, 3 * C], f32, tag="qkvp")
                    nc.tensor.matmul(qkvp[:], lhsT=hT[:, j * RPB:(j + 1) * RPB],
                                     rhs=wqkvT[:], start=True, stop=True)
                    nc.vector.tensor_tensor(out=qkv[:, j, :], in0=qkvp[:],
                                            in1=qkvb_rep[:RPB, :], op=AL.add)

                # rope, batched over blocks via strided views
                ta = sb.tile([RPB, BPG * 64], f32)
                tb = sb.tile([RPB, BPG * 64], f32)
                tcs = sb.tile([RPB, BPG * 64], f32)

                def halves(base):
                    x1 = bass.AP(tensor=qkv.tensor, offset=qkv[:, 0, base].offset,
                                 ap=[qkv[:, 0, 0].ap[0],
                                     [3 * C, BPG], [D, H], [1, D // 2]])
                    x2 = bass.AP(tensor=qkv.tensor,
                                 offset=qkv[:, 0, base + D // 2].offset,
                                 ap=[qkv[:, 0, 0].ap[0],
                                     [3 * C, BPG], [D, H], [1, D // 2]])
                    return x1, x2

                def cs_view(col0):
                    return bass.AP(tensor=cst.tensor,
                                   offset=cst[:, col0].offset,
                                   ap=[cst[:, 0].ap[0], [16, BPG], [0, H],
                                       [1, D // 2]])

                def t3(t):
                    return bass.AP(tensor=t.tensor, offset=t[:, 0].offset,
                                   ap=[t[:, 0].ap[0], [64, BPG], [8, H], [1, 8]])

                cb, sbv = cs_view(0), cs_view(8)
                for base in (0, C):
                    x1, x2 = halves(base)
                    nc.vector.tensor_tensor(out=t3(ta), in0=x1, in1=cb, op=AL.mult)
                    nc.vector.tensor_tensor(out=t3(tb), in0=x1, in1=sbv, op=AL.mult)
                    nc.vector.tensor_tensor(out=t3(tcs), in0=x2, in1=sbv, op=AL.mult)
                    nc.vector.tensor_tensor(out=x1, in0=t3(ta), in1=t3(tcs), op=AL.subtract)
                    nc.vector.tensor_tensor(out=t3(ta), in0=x2, in1=cb, op=AL.mult)
                    nc.vector.tensor_tensor(out=x2, in0=t3(ta), in1=t3(tb), op=AL.add)

                # transpose q,k to T layout; build v1 = [V|1] per head
                qT = sb.tile([128, RPG], f32)
                kT = sb.tile([128, RPG], f32)
                v1 = sb.tile([RPB, BPG, H * (D + 1)], f32)
                for j in range(BPG):
                    tq = pp.tile([128, RPB], f32, tag="tr")
                    nc.tensor.transpose(tq[:], qkv[:, j, 0:C], ident[:RPB, :RPB])
                    nc.vector.tensor_copy(out=qT[:, j * RPB:(j + 1) * RPB], in_=tq[:])
                    tk = pp.tile([128, RPB], f32, tag="tr")
                    nc.tensor.transpose(tk[:], qkv[:, j, C:2 * C], ident[:RPB, :RPB])
                    nc.vector.tensor_copy(out=kT[:, j * RPB:(j + 1) * RPB], in_=tk[:])
                    vdst = bass.AP(tensor=v1.tensor, offset=v1[:, j, 0].offset,
                                   ap=[v1[:, 0, 0].ap[0], [D + 1, H], [1, D]])
                    vsrc = bass.AP(tensor=qkv.tensor, offset=qkv[:, j, 2 * C].offset,
                                   ap=[qkv[:, 0, 0].ap[0], [D, H], [1, D]])
                    nc.vector.tensor_copy(out=vdst, in_=vsrc)
                    onesd = bass.AP(tensor=v1.tensor, offset=v1[:, j, D].offset,
                                    ap=[v1[:, 0, 0].ap[0], [D + 1, H], [1, 1]])
                    nc.gpsimd.memset(onesd, 1.0)

                qTh = sb.tile([128, H, RPG], f32)
                for h in range(H):
                    nc.vector.tensor_scalar_mul(out=qTh[:, h, :], in0=qT[:],
                                                scalar1=mh[:, h:h + 1])

                # attention per block
                osb = sb.tile([RPB, BPG, C], bf)
                for j in range(BPG):
                    expS = sb.tile([RPB, H, RPB], f32)
                    for m in range(2):
                        sB = pp.tile([RPB, 4 * RPB], f32, tag="sB%d" % m)
                        for hh in range(4):
                            h = 4 * m + hh
                            nc.tensor.matmul(
                                sB[:, hh * RPB:(hh + 1) * RPB],
                                lhsT=kT[:, j * RPB:(j + 1) * RPB],
                                rhs=qTh[:, h, j * RPB:(j + 1) * RPB],
                                start=True, stop=True)
                        etmp = sb.tile([RPB, 4 * RPB], f32, tag="etmp")
                        nc.scalar.activation(out=etmp[:], in_=sB[:], func=AF.Exp)
                        mview = bass.AP(tensor=bdm.tensor, offset=bdm[:, 0].offset,
                                        ap=[bdm[:, 0].ap[0], [0, 4], [1, RPB]])
                        nc.vector.tensor_tensor(
                            out=expS[:, 4 * m:4 * (m + 1), :],
                            in0=etmp[:].rearrange("p (h r) -> p h r", h=4),
                            in1=mview, op=AL.mult)
                    oP = pp.tile([RPB, H * (D + 1)], f32, tag="oP")
                    for h in range(H):
                        nc.tensor.matmul(
                            oP[:, h * (D + 1):(h + 1) * (D + 1)],
                            lhsT=expS[:, h, :],
                            rhs=v1[:, j, h * (D + 1):(h + 1) * (D + 1)],
                            start=True, stop=True)
                    rec = sb.tile([RPB, H], f32)
                    dview = bass.AP(tensor=oP.tensor, offset=oP[:, D].offset,
                                    ap=[oP[:, 0].ap[0], [D + 1, H], [1, 1]])
                    nc.vector.reciprocal(out=rec[:].rearrange("p (h o) -> p h o", h=H),
                                         in_=dview)
                    oview = bass.AP(tensor=oP.tensor, offset=oP[:, 0].offset,
                                    ap=[oP[:, 0].ap[0], [D + 1, H], [1, D]])
                    rview = bass.AP(tensor=rec.tensor, offset=rec[:, 0].offset,
                                    ap=[rec[:, 0].ap[0], [1, H], [0, D]])
                    nc.vector.tensor_tensor(
                        out=osb[:, j, :].rearrange("p (h d) -> p h d", h=H),
                        in0=oview, in1=rview, op=AL.mult)

                # contiguous stripe write: stripe row = g*RPG + j*RPB + p
                nc.sync.dma_start(
                    out=stripe[ts(g, RPG)].rearrange("(j p) c -> p j c", j=BPG),
                    in_=osb[:, :, :])

            if not os.environ.get("SKIP_P1"):
                if FORI:
                    with tc.For_i(0, G1, 1) as gv:
                        p1_body(gv)
                else:
                    for g in range(G1):
                        p1_body(g)
            pp1_cm.__exit__(None, None, None)

            # ---------------- AllGather ----------------
            nc.gpsimd.collective_compute(
                "AllGather", AL.bypass,
                replica_groups=[list(range(NCORES))],
                ins=[stripe[:, :]], outs=[table[:, :]])

            # ---------------- Phase 2 ----------------
            pp2_cm = tc.tile_pool(name="pp2", bufs=2, space="PSUM")
            pp2 = pp2_cm.__enter__()

            def p2_body(g):
                g2o = sb.tile([128, W2], i32)
                nc.sync.dma_start(out=g2o[:], in_=gofs2_d[ts(g, 128)])
                tid = sb.tile([128, W2 + 1], bf)
                nc.sync.dma_start(out=tid[:], in_=tids_d[ts(g, 128)])
                gath = sb.tile([128, W2, C], bf)
                for j in range(W2):
                    nc.gpsimd.indirect_dma_start(
                        out=gath[:, j, :], out_offset=None, in_=table[:, :],
                        in_offset=bass.IndirectOffsetOnAxis(ap=g2o[:, j:j + 1], axis=0))

                binp = pp2.tile([128, C], f32, tag="mm2")
                for j in range(W2):
                    oh = sb.tile([128, 128], bf, tag="oh")
                    nc.vector.tensor_tensor(out=oh[:], in0=tid[:, j:j + 1].to_broadcast([128, 128]),
                                            in1=iota_sb[:], op=AL.is_equal)
                    nc.tensor.matmul(binp[:], lhsT=oh[:], rhs=gath[:, j, :],
                                     start=(j == 0), stop=(j == W2 - 1))
                invf = sb.tile([128, 1], f32, tag="invf")
                nc.vector.tensor_copy(out=invf[:], in_=tid[:, W2:W2 + 1])
                a_sb = sb.tile([128, C], f32)
                nc.vector.tensor_scalar_mul(out=a_sb[:], in0=binp[:],
                                            scalar1=invf[:])
                tpa = pp2.tile([128, 128], f32, tag="tr2")
                nc.tensor.transpose(tpa[:], a_sb[:], ident[:])
                aT = sb.tile([128, 128], f32)
                nc.vector.tensor_copy(out=aT[:], in_=tpa[:])
                prp = pp2.tile([128, C], f32, tag="mm2")
                nc.tensor.matmul(prp[:], lhsT=aT[:], rhs=projwT[:], start=True, stop=True)

                asum = sb.tile([128, C], f32)
                nc.vector.tensor_tensor(out=asum[:], in0=prp[:], in1=projb_rep[:], op=AL.add)
                feattb = sb.tile([128, C], f8)
                nc.sync.dma_start(out=feattb[:], in_=featsh_d[ts(g, 128)])
                featt = sb.tile([128, C], f32)
                nc.vector.tensor_copy(out=featt[:], in_=feattb[:])
                x_sb = sb.tile([128, C], f32)
                nc.vector.tensor_tensor(out=x_sb[:], in0=asum[:], in1=featt[:], op=AL.add)

                # LN2
                s1 = sb.tile([128, 1], f32, tag="s1")
                nc.vector.reduce_sum(out=s1[:], in_=x_sb[:], axis=AX)
                mn = sb.tile([128, 1], f32, tag="mn")
                nc.scalar.activation(out=mn[:], in_=s1[:], func=AF.Copy, bias=0.0,
                                     scale=1.0 / C)
                sqx = sb.tile([128, C], f32, tag="sqx")
                nc.vector.tensor_tensor(out=sqx[:], in0=x_sb[:], in1=x_sb[:], op=AL.mult)
                s2 = sb.tile([128, 1], f32, tag="s2")
                nc.vector.reduce_sum(out=s2[:], in_=sqx[:], axis=AX)
                e2 = sb.tile([128, 1], f32, tag="e2")
                nc.scalar.activation(out=e2[:], in_=s2[:], func=AF.Copy, bias=0.0,
                                     scale=1.0 / C)
                m2t = sb.tile([128, 1], f32, tag="m2t")
                nc.vector.tensor_tensor(out=m2t[:], in0=mn[:], in1=mn[:], op=AL.mult)
                vr = sb.tile([128, 1], f32, tag="vr")
                nc.vector.tensor_tensor(out=vr[:], in0=e2[:], in1=m2t[:], op=AL.subtract)
                rs = sb.tile([128, 1], f32, tag="rs")
                nc.scalar.activation(out=rs[:], in_=vr[:], func=AF.Sqrt,
                                     bias=eps128[:], scale=1.0)
                nc.vector.reciprocal(out=rs[:], in_=rs[:])
                h2 = sb.tile([128, C], f32, tag="h2")
                nc.vector.tensor_scalar(out=h2[:], in0=x_sb[:], scalar1=mn[:],
                                        scalar2=rs[:], op0=AL.subtract, op1=AL.mult)
                tph = pp2.tile([128, 128], f32, tag="tr2")
                nc.tensor.transpose(tph[:], h2[:], ident[:])
                h2T = sb.tile([128, 128], f32, tag="h2T")
                nc.vector.tensor_copy(out=h2T[:], in_=tph[:])

                gT = sb.tile([128, 4, 128], f32, tag="gT")
                for j in range(4):
                    f1p = pp2.tile([128, 128], f32, tag="f1p")
                    nc.tensor.matmul(f1p[:], lhsT=fc1wT[:, j * 128:(j + 1) * 128],
                                     rhs=h2T[:], start=True, stop=True)
                    nc.scalar.activation(out=gT[:, j, :], in_=f1p[:], func=AF.Gelu,
                                         bias=fc1bias[:, j:j + 1], scale=1.0)
                f2p = pp2.tile([128, C], f32, tag="mm2")
                for j in range(4):
                    nc.tensor.matmul(f2p[:], lhsT=gT[:, j, :], rhs=fc2wTT[:, j, :],
                                     start=(j == 0), stop=(j == 3))
                dsb = sb.tile([128, C], f8, tag="dsb")
                nc.vector.tensor_tensor(out=dsb[:], in0=f2p[:], in1=asum[:], op=AL.add)
                nc.vector.tensor_tensor(out=dsb[:], in0=dsb[:], in1=fc2b_rep[:], op=AL.add)
                nc.sync.dma_start(out=delta_d[ts(g, 128)], in_=dsb[:])

            if not os.environ.get("SKIP_P2"):
                if FORI:
                    with tc.For_i(0, NG2, 1) as gv2:
                        p2_body(gv2)
                else:
                    for g in range(NG2):
                        p2_body(g)
            else:
                zt = sb.tile([128, C], f8, tag="dsb")
                nc.vector.memset(zt[:], 0.0)
                for g in range(NG2):
                    nc.sync.dma_start(out=delta_d[ts(g, 128)], in_=zt[:])
            pp2_cm.__exit__(None, None, None)

    nc.compile()
    return nc


def _import_jax():
    import jax
    from concourse import bass2jax  # noqa: F401
    try:
        devs = jax.devices()
        # Warm backend init + the axon transfer path (no compilation).
        jax.device_put(np.ones((8, 8), np.float32), devs[0]).block_until_ready()
        import libneuronxla  # noqa: F401
    except Exception:
        pass


def _import_worker():
    # Warm the heavy imports while the main thread does numpy index prep,
    # then trace+schedule a tiny throwaway program so the first real build
    # hits warm code paths (bass_rust engines, tile scheduler, masks, ...).
    import concourse.bass_utils  # noqa: F401
    import concourse.bacc as bacc
    import concourse.tile as tile
    from concourse import mybir  # noqa: F401
    from concourse.masks import make_identity
    nc = bacc.Bacc("TRN2", target_bir_lowering=False, num_devices=NCORES)
    x_d = nc.dram_tensor("x", [128, 128], mybir.dt.float32, kind="ExternalInput")
    y_d = nc.dram_tensor("y", [128, 128], mybir.dt.float32, kind="ExternalOutput")
    with tile.TileContext(nc) as tc:
        with tc.tile_pool(name="sb", bufs=1) as sb:
            t = sb.tile([128, 128], mybir.dt.float32)
            make_identity(nc, t[:])
            nc.sync.dma_start(out=t[:], in_=x_d[:, :])
            with tc.For_i(0, 2, 1) as i:
                from concourse.bass import ts
                t2 = sb.tile([64, 128], mybir.dt.float32)
                nc.vector.tensor_scalar_mul(out=t2[:], in0=t[0:64, :], scalar1=t[0:64, 0:1])
                nc.sync.dma_start(out=y_d[ts(i, 64), :], in_=t2[:])
            nc.sync.dma_start(out=y_d[:, :], in_=t[:])
    nc.compile()
    nc.to_json_bytes()


def kernel(**inputs):
    import threading
    import time as _time
    _t0 = _time.time()
    _tlog = (lambda msg: print(f"[ktime] {msg}: {_time.time()-_t0:.2f}s", flush=True)) \
        if os.environ.get("KTIME") else (lambda msg: None)
    _imp = threading.Thread(target=_import_worker)
    _imp.start()
    _impj = threading.Thread(target=_import_jax)
    _impj.start()
    _tlog("import threads started")

    feat = np.ascontiguousarray(np.asarray(inputs["feat"], dtype=np.float32))
    cos = np.asarray(inputs["cos"], dtype=np.float32)
    sin = np.asarray(inputs["sin"], dtype=np.float32)
    pad = np.asarray(inputs["pad"]).astype(np.int64)
    unpad = np.asarray(inputs["unpad"]).astype(np.int64)
    pinv = np.asarray(inputs["path_inverse"]).astype(np.int64)
    H = int(inputs["num_heads"])
    K = int(inputs["patch_size"])
    N, C = feat.shape
    D = C // H
    M = pinv.shape[0]
    Mpad = pad.shape[0]
    Wn = Mpad // K
    assert Wn * K == Mpad and N % (128 * NCORES) == 0

    BW, BPG = 2, 2
    RPB = BW * K
    RPG = BPG * RPB
    WPG = BW * BPG                       # windows per phase-1 group
    wpc = (Wn + NCORES - 1) // NCORES    # windows per core
    G1 = (wpc + WPG - 1) // WPG
    SROWS = G1 * RPG
    NSL = N // NCORES
    NG2 = NSL // 128
    NGRP = N // 128

    gidx_all = pinv[pad]                           # [Mpad] -> feat row
    real_all = unpad[pad] == np.arange(Mpad)       # row is a real (unpadded) row

    # ---- phase-1 grids (vectorized) ----
    c_ = np.arange(NCORES)[:, None, None, None]
    g_ = np.arange(G1)[None, :, None, None]
    p_ = np.arange(RPB)[None, None, :, None]
    j_ = np.arange(BPG)[None, None, None, :]
    s_ = g_ * WPG + j_ * BW + p_ // K              # slot in core
    w_ = c_ * wpc + s_                             # global window
    mask = (s_ < wpc) & (w_ < Wn)
    mp_ = np.where(mask, w_ * K + p_ % K, 0)
    gofs = np.where(mask, gidx_all[mp_], 0).astype(np.int32)
    pid_ = pad[mp_]
    cos8 = np.ascontiguousarray(cos[:, :D // 2])
    sin8 = np.ascontiguousarray(sin[:, :D // 2])
    csq = np.empty((NCORES, G1, RPB, BPG, 16), np.float32)
    csq[..., 0:8] = cos8[pid_]
    csq[..., 8:16] = sin8[pid_]
    csq *= mask[..., None]
    _tlog("host p1 prep")

    # ---- phase-2: per 128-point group source lists ----
    mp_real = np.nonzero(real_all)[0]
    wr = mp_real // K
    rr = mp_real - wr * K
    cr = wr // wpc
    sr = wr - cr * wpc
    jjr = sr % WPG
    srow = (sr // WPG) * RPG + (jjr // BW) * RPB + (jjr % BW) * K + rr
    tblpos = cr * SROWS + srow
    tgt = gidx_all[mp_real]
    order = np.argsort(tgt, kind="stable")
    tgt_s, tbl_s = tgt[order], tblpos[order]
    starts = np.searchsorted(tgt_s, np.arange(0, N + 1, 128))
    cnt_grp = np.diff(starts)
    W2 = int(math.ceil(cnt_grp.max() / 128))
    grp = tgt_s // 128
    off = np.arange(tgt_s.shape[0]) - starts[grp]
    gof_flat = np.zeros((NGRP, 128 * W2), np.int32)
    tid_flat = np.full((NGRP, 128 * W2), -1.0, np.float32)
    gof_flat[grp, off] = tbl_s
    tid_flat[grp, off] = (tgt_s - grp * 128).astype(np.float32)
    gofs2 = gof_flat.reshape(NCORES, NG2 * 128, W2)
    cnts = np.bincount(pinv, minlength=N)
    invc = (1.0 / np.maximum(cnts, 1)).astype(np.float32)
    tids2 = np.concatenate(
        [tid_flat.reshape(NGRP, 128, W2), invc.reshape(NGRP, 128, 1)],
        axis=2).reshape(NCORES, NG2 * 128, W2 + 1)
    import ml_dtypes
    tids2 = tids2.astype(ml_dtypes.bfloat16)
    _tlog("host p2 prep")

    # ---- weights (LN affines + q-scale folded on host) ----
    g1 = np.asarray(inputs["g1"], np.float32)
    b1 = np.asarray(inputs["b1"], np.float32)
    g2 = np.asarray(inputs["g2"], np.float32)
    b2 = np.asarray(inputs["b2"], np.float32)
    qkv_w = np.asarray(inputs["qkv_w"], np.float32)
    qkv_b = np.asarray(inputs["qkv_b"], np.float32)
    proj_w = np.asarray(inputs["proj_w"], np.float32)
    fc1_w = np.asarray(inputs["fc1_w"], np.float32)
    fc1_b = np.asarray(inputs["fc1_b"], np.float32)
    fc2_w = np.asarray(inputs["fc2_w"], np.float32)
    qs = float(D) ** -0.5
    wqkvT = qkv_w.T * g1[:, None]
    wqkvT[:, 0:C] *= qs
    qkvb = b1 @ qkv_w.T + qkv_b
    qkvb[0:C] *= qs
    fc1wT = fc1_w.T * g2[:, None]
    fc1b = b2 @ fc1_w.T + fc1_b
    fc2wTT = np.stack([np.ascontiguousarray(fc2_w[:, j * 128:(j + 1) * 128].T)
                       for j in range(4)])
    fc2wTT_perm = np.ascontiguousarray(fc2wTT.transpose(1, 0, 2))  # [128, 4, C]
    mh_ = (np.arange(128)[:, None] // D == np.arange(H)[None, :]).astype(np.float32)
    bdmask_ = np.kron(np.eye(BW, dtype=np.float32), np.ones((K, K), np.float32))
    wblob = np.concatenate([
        np.ascontiguousarray(wqkvT).ravel(),
        np.ascontiguousarray(proj_w.T).ravel(),
        np.ascontiguousarray(fc1wT).ravel(),
        fc2wTT_perm.ravel(),
        qkvb.astype(np.float32).ravel(),
        np.asarray(inputs["proj_b"], np.float32).ravel(),
        np.asarray(inputs["fc2_b"], np.float32).ravel(),
        np.ascontiguousarray(fc1b.reshape(4, 128).T).ravel(),
        mh_.ravel(),
        bdmask_.ravel(),
    ]).astype(np.float32)
    wblob = np.concatenate([wblob, np.zeros(((-len(wblob)) % NCORES,), np.float32)])
    wsh = wblob.reshape(NCORES, -1)
    common = {
        "iota": np.tile(np.arange(128, dtype=np.float32), (128, 1)).astype(ml_dtypes.bfloat16),
    }
    shapes = dict(N=N, C=C, H=H, D=D, K=K, G1=G1, NG2=NG2, W2=W2, BPG=BPG)
    import ml_dtypes
    bf16 = ml_dtypes.bfloat16
    f8np = ml_dtypes.float8_e4m3
    featbf = feat.astype(f8np)
    csqbf = csq.astype(f8np)
    _tlog("input prep")
    _imp.join()
    _impj.join()
    _tlog("import join")
    from concourse.bass_utils import run_bass_kernel_spmd
    nc = _build_program(shapes)
    _tlog("build+compile bass")
    in_maps = []
    for c in range(NCORES):
        m = dict(common)
        m.update({
            "featsh": featbf[c * NSL:(c + 1) * NSL],
            "wsh": wsh[c],
            "cs": csqbf[c].reshape(G1 * RPB, BPG * 16),
            "gofs": gofs[c].reshape(G1 * RPB, BPG),
            "gofs2": gofs2[c], "tids": tids2[c],
        })
        in_maps.append(m)
    _tlog("in_maps")
    res = run_bass_kernel_spmd(nc, in_maps, core_ids=list(range(NCORES)))
    _tlog("run_bass_kernel_spmd")
    outs = res.results
    y = np.empty_like(feat)
    for c in range(NCORES):
        sl = slice(c * NSL, (c + 1) * NSL)
        np.add(feat[sl], np.asarray(outs[c]["delta"], dtype=np.float32), out=y[sl])
    _tlog("gather output")
    return y


if __name__ == "__main__":
    sys.path.insert(0, "/root/problem")
    import reference
    inp = reference.setup_inputs()
    inp = {k: np.asarray(v) if hasattr(v, "shape") else v for k, v in inp.items()}
    out = kernel(**inp)
    print("kernel out", out.shape, out.dtype)
